# revision 44
# baseline (speedup 1.0000x reference)
"""Baichuan attention layer (B=1, S=2048, E=4096, H=32, D=128) on 8 Trainium2
NeuronCores.

Sharding:
- QKV projection + RoPE + causal attention: tensor-parallel by head (4 heads
  per core).  q/k live in transposed [feature, seq] layout so the RoPE and
  score matmuls contract over the partition dim with zero transposes:
    qk^T[f, s]    = W @ X^T                (lhsT = W^T tiles, rhs = X^T tiles)
    scores^T[k,q] = K @ Q^T                (lhsT = K^T tile, rhs = Q^T block)
    att^T[d, q]   = V^T @ P^T              (lhsT = V tile,   rhs = exp tile)
  v is computed directly in [seq, (head d)] layout by swapping matmul
  operands (lhsT = X^T s-tile, rhs = Wv^T block) so no PE transposes are
  needed before attention.  Softmax runs without max-subtraction (scores ~
  N(0,1) after 1/sqrt(D), fp32 exp is safe); the denominator is accumulated
  with an all-ones [128,128] lhsT matmul so the k-sum lands in PSUM already
  replicated across partitions.  The causal-mask multiply runs on the (idle)
  GpSimd engine to keep the Vector queue short.
- One AllGather of att^T [512, 2048] bf16 per core -> full att^T [4096, 2048].
- o_proj: column-parallel (each core computes its 512 output columns for the
  full sequence, using its slice of w_o).  Host concatenates along E.

All matmuls in bf16 with fp32 PSUM accumulation.  All DRAM operands are
host-packed so every DMA line is 2KB-32KB contiguous per partition (the
naive layouts produced 256B descriptors that throttled the DMA queues).
"""

import importlib.util
import sys
import types

import numpy as np
import ml_dtypes

BF16NP = ml_dtypes.bfloat16

B, S, E = 1, 2048, 4096
H, D = 32, 128
NCORES = 8
HPC = H // NCORES          # heads per core = 4
P = 128                    # partitions
SBLK = 512                 # seq block (matmul free dim)
NSBLK = S // SBLK          # 4
ET = E // P                # 32 e-tiles
NQK = 2 * HPC              # 8 q/k f-tiles per core (q0..3, k0..3)
KT = S // P                # 16 k-tiles
ECOLS = E // NCORES        # 512 output columns per core
VF = HPC * D               # 512 v feature columns per core
SCALE = 1.0 / float(np.sqrt(D))


def _install_ntff_hook():
    """antenv.axon_hooks is absent in this image; recreate it from trn_boot's
    ctypes shim so run_bass_kernel_spmd(trace=True) can capture NTFF traces."""
    if "antenv.axon_hooks" in sys.modules:
        return
    try:
        spec = importlib.util.spec_from_file_location(
            "trn_boot", "/root/.axon_site/trn_agent_boot/trn_boot.py")
        tb = importlib.util.module_from_spec(spec)
        spec.loader.exec_module(tb)
        hook = tb._ntff_profile_via_ctypes("/opt/axon/libaxon_pjrt.so")
    except Exception:
        hook = None
    mod = types.ModuleType("antenv.axon_hooks")
    mod.get_axon_ntff_profile_hook = lambda: hook
    mod.set_axon_ntff_profile_hook = lambda h: None
    sys.modules["antenv.axon_hooks"] = mod


_install_ntff_hook()

import concourse.bass as bass  # noqa: E402
import concourse.mybir as mybir  # noqa: E402
import concourse.tile as tile  # noqa: E402
from concourse import bacc  # noqa: E402
from concourse.bass import ts  # noqa: E402
from concourse.bass_utils import run_bass_kernel_spmd  # noqa: E402

BF16 = mybir.dt.bfloat16
F32 = mybir.dt.float32

_NC_CACHE = None


def build():
    global _NC_CACHE
    if _NC_CACHE is not None:
        return _NC_CACHE
    nc = bacc.Bacc("TRN2", target_bir_lowering=False, debug=False,
                   num_devices=NCORES)

    # Host-packed layouts: partition dim explicit, per-partition lines long.
    xt_ext = nc.dram_tensor("xt", [NSBLK, P, ET, SBLK], BF16,
                            kind="ExternalInput")
    wqk_ext = nc.dram_tensor("wqk", [NQK, P, ET, P], BF16,
                             kind="ExternalInput")
    wv_ext = nc.dram_tensor("wv", [P, ET, VF], BF16, kind="ExternalInput")
    wot_ext = nc.dram_tensor("wot", [P, ET, ECOLS], BF16,
                             kind="ExternalInput")
    cost_ext = nc.dram_tensor("cost", [D, S], F32, kind="ExternalInput")
    # sin with rows 0..63 negated: rope is q*cos + swap(q)*sin_signed where
    # swap is the rt permutation matmul
    sint_ext = nc.dram_tensor("sint", [D, S], F32, kind="ExternalInput")
    rt_ext = nc.dram_tensor("rt", [D, D], BF16, kind="ExternalInput")
    masks_ext = nc.dram_tensor("masks", [4, P, SBLK], BF16,
                               kind="ExternalInput")
    ones_ext = nc.dram_tensor("ones", [P, P], BF16, kind="ExternalInput")
    out_ext = nc.dram_tensor("out", [S, ECOLS], F32, kind="ExternalOutput")

    # Two AllGathers, one per pair of local heads (01 issued after head 1,
    # 23 after head 3): fewer collective triggers shorten the serial CC
    # chain that gates the late o_proj passes.  ccout pair rank-r block:
    # rows [256r + 128q, ...) = global head 4r + (2*pair + q).
    ccins = [nc.dram_tensor(f"ccin{g}", [2 * P, S], BF16) for g in range(2)]
    ccouts = [nc.dram_tensor(f"ccout{g}", [NCORES * 2 * P, S], BF16,
                             addr_space="Shared") for g in range(2)]

    xt_t = xt_ext.ap().rearrange("b p e s -> p b e s")
    wqk_t = wqk_ext.ap().rearrange("f p e c -> p f e c")
    masks_t = masks_ext.ap().rearrange("r p q -> p r q")
    # [p, q(two), r(rank), s]: head of (pair g, q, rank r) = 4r + 2g + q
    ccout_ts = [cc.ap().rearrange("(r two p) s -> p two r s", two=2, p=P)
                for cc in ccouts]

    with tile.TileContext(nc) as tc:
        with (
            tc.tile_pool(name="cst", bufs=1) as cst,
            tc.tile_pool(name="ropeT", bufs=1) as ropeT_pool,
            tc.tile_pool(name="vall", bufs=1) as vall_pool,
        ):
            rt_sb = cst.tile([D, D], BF16)
            ones_sb = cst.tile([P, P], BF16)
            masks_sb = cst.tile([P, 4, SBLK], BF16)

            # q^T and k^T after RoPE: [128, 8, 2048]
            ropeT_sb = ropeT_pool.tile([P, NQK, S], BF16)
            # V in [s, (h d)] layout: [128, 16 k-tiles, 512]
            v_all_sb = vall_pool.tile([P, KT, VF], BF16)

            # ---------------- Phase 1: QKV projection + RoPE -------------
            with (
                tc.tile_pool(name="xt", bufs=2) as xt_pool,
                tc.tile_pool(name="wqk", bufs=3) as w_pool,
                tc.tile_pool(name="wv", bufs=1) as wv_pool,
                tc.tile_pool(name="cs", bufs=1) as cs_pool,
                tc.tile_pool(name="qkc", bufs=3) as qkc_pool,
                tc.tile_pool(name="rtmp", bufs=3) as rtmp_pool,
                tc.tile_pool(name="ps_qkv", bufs=4, space="PSUM") as ps_qkv,
                tc.tile_pool(name="ps_rot", bufs=2, space="PSUM") as ps_rot,
                tc.tile_pool(name="ps_v", bufs=2, space="PSUM") as ps_v,
            ):
                cos_sb = cs_pool.tile([D, S], F32)
                sin_sb = cs_pool.tile([D, S], F32)
                wv_sb = wv_pool.tile([P, ET, VF], BF16)

                def v_groups(vb, xt_tile):
                    # v for block vb, directly in [s, (h d)] layout:
                    # out[s128, f512] = sum_e xt[e, s-tile].T @ wv[e-tile, :]
                    for st in range(NSBLK):
                        v_ps = ps_v.tile([P, VF], F32, tag="v")
                        for e in range(ET):
                            nc.tensor.matmul(
                                v_ps[:], xt_tile[:, e, ts(st, P)],
                                wv_sb[:, e, :],
                                start=(e == 0), stop=(e == ET - 1),
                            )
                        nc.any.tensor_copy(v_all_sb[:, 4 * vb + st, :],
                                           v_ps[:])

                prev_xt = None
                for b in range(NSBLK):
                    sblk = ts(b, SBLK)
                    xt_sb = xt_pool.tile([P, ET, SBLK], BF16, tag="xt")
                    w0_sb = None
                    if b == 0:
                        # tiny interleaved leading chunks: the first matmuls
                        # need only (xt e0-1, w0 e0-3), so the PE can start
                        # right after its preamble
                        w0_sb = w_pool.tile([P, ET, P], BF16, tag="w")
                        xs = [2, 2, 4, 4, 4, 4, 4, 4, 4]
                        wsz = [4, 4, 8, 16]
                        xo = [sum(xs[:i]) for i in range(len(xs))]
                        wo = [sum(wsz[:i]) for i in range(len(wsz))]
                        seq = []
                        for i in range(len(xs)):
                            seq.append(("x", xo[i], xs[i]))
                            if i < len(wsz):
                                seq.append(("w", wo[i], wsz[i]))
                        for kind, e0, sz in seq:
                            if kind == "x":
                                nc.sync.dma_start(
                                    xt_sb[:, bass.ds(e0, sz), :],
                                    xt_t[:, b, bass.ds(e0, sz), :])
                            else:
                                nc.sync.dma_start(
                                    w0_sb[:, bass.ds(e0, sz), :],
                                    wqk_t[:, 0, bass.ds(e0, sz), :])
                    else:
                        for ch in range(4):
                            nc.sync.dma_start(xt_sb[:, ts(ch, 8), :],
                                              xt_t[:, b, ts(ch, 8), :])

                    for f in range(NQK):
                        if b == 0 and f == 0:
                            w_sb = w0_sb
                        else:
                            w_sb = w_pool.tile([P, ET, P], BF16, tag="w")
                            for ch in range(2):
                                nc.sync.dma_start(
                                    w_sb[:, ts(ch, 16), :],
                                    wqk_t[:, f, ts(ch, 16), :])
                        if b == 0 and f == 0:
                            # only what the first rope mul needs; wv and the
                            # attention consts load in later, calmer windows
                            nc.sync.dma_start(rt_sb[:], rt_ext.ap())
                            nc.sync.dma_start(cos_sb[:], cost_ext.ap())
                            nc.sync.dma_start(sin_sb[:], sint_ext.ap())
                        if b == 0 and f == 4:
                            for ch in range(4):
                                nc.sync.dma_start(
                                    wv_sb[:, ts(ch, ET // 4), :],
                                    wv_ext.ap()[:, ts(ch, ET // 4), :])
                        if b == 2 and f == 0:
                            nc.sync.dma_start(ones_sb[:], ones_ext.ap())
                            nc.sync.dma_start(masks_sb[:], masks_t)
                        acc_ps = ps_qkv.tile([P, SBLK], F32, tag="qkv")
                        for e in range(ET):
                            nc.tensor.matmul(
                                acc_ps[:], w_sb[:, e, :], xt_sb[:, e, :],
                                start=(e == 0), stop=(e == ET - 1),
                            )
                        # RoPE: rot = R @ qk on the PE (sign of rotate-half
                        # folded into sint on the host; rt is a pure swap
                        # permutation)
                        qk_sb = qkc_pool.tile([P, SBLK], BF16, tag="qkc")
                        nc.any.tensor_copy(qk_sb[:], acc_ps[:])
                        rot_ps = ps_rot.tile([P, SBLK], F32, tag="rot")
                        nc.tensor.matmul(rot_ps[:], rt_sb[:], qk_sb[:],
                                         start=True, stop=True)
                        t1 = rtmp_pool.tile([P, SBLK], F32, tag="t1")
                        nc.vector.tensor_mul(out=t1[:], in0=acc_ps[:],
                                             in1=cos_sb[:, sblk])
                        t2 = rtmp_pool.tile([P, SBLK], F32, tag="t2")
                        nc.vector.tensor_mul(out=t2[:], in0=rot_ps[:],
                                             in1=sin_sb[:, sblk])
                        nc.vector.tensor_add(
                            out=ropeT_sb[:, f, sblk], in0=t1[:], in1=t2[:])

                    # v of the PREVIOUS block: its xt is still resident and
                    # this keeps block 0's startup window free of wv traffic
                    if b > 0:
                        v_groups(b - 1, prev_xt)
                    if b == NSBLK - 1:
                        v_groups(b, xt_sb)
                    prev_xt = xt_sb

            # ------------- Phase 2: attention per head + AllGather -------
            with (
                tc.tile_pool(name="wot", bufs=1) as wot_pool,
                tc.tile_pool(name="attnT", bufs=1) as attnT_pool,
                tc.tile_pool(name="slab", bufs=4) as slab_pool,
            ):
                wot_sb = wot_pool.tile([P, ET, ECOLS], BF16)
                for ch in range(4):
                    nc.sync.dma_start(wot_sb[:, ts(ch, ET // 4), :],
                                      wot_ext.ap()[:, ts(ch, ET // 4), :])
                attnT_sb = attnT_pool.tile([P, HPC, S], BF16)

                # o_proj input staged as half-slabs (st 0-7 / st 8-15 of one
                # gathered head-group) so the WAR gate on buffer reuse opens
                # at half-pass granularity and transfers start sooner
                slabs = {}

                def load_slab(p_h, half):
                    a_sb = slab_pool.tile([P, NCORES, S // 2], BF16,
                                          tag="slab")
                    s0 = half * (S // 2)
                    src = ccout_ts[p_h // 2]
                    for cc in range(4):
                        nc.sync.dma_start(
                            a_sb[:, bass.ds(2 * cc, 2), :],
                            src[:, p_h % 2, bass.ds(2 * cc, 2),
                                bass.ds(s0, S // 2)])
                    slabs[(p_h, half)] = a_sb

                with (
                    tc.tile_pool(name="exp", bufs=8) as exp_pool,
                    tc.tile_pool(name="rcp", bufs=2) as rcp_pool,
                    tc.tile_pool(name="ps_sc", bufs=4, space="PSUM") as ps_sc,
                    tc.tile_pool(name="ps_av", bufs=2, space="PSUM") as ps_av,
                    tc.tile_pool(name="ps_den", bufs=2, space="PSUM") as ps_den,
                ):
                    for h in range(HPC):
                        qh = ropeT_sb[:, h, :]
                        kh = ropeT_sb[:, HPC + h, :]
                        vh = v_all_sb[:, :, ts(h, D)]
                        for j in range(NSBLK):
                            nkt = 4 * j + 4
                            av_ps = ps_av.tile([P, SBLK], F32, tag="av")
                            den_ps = ps_den.tile([P, SBLK], F32, tag="den")
                            for i in range(nkt):
                                # diagonal tile r: columns below 128r are
                                # fully masked -> compute only [off:SBLK]
                                r = i - 4 * j
                                off = 128 * r if r > 0 else 0
                                qs = bass.ds(j * SBLK + off, SBLK - off)
                                sc_ps = ps_sc.tile([P, SBLK], F32, tag="sc")
                                nc.tensor.matmul(sc_ps[:, off:],
                                                 kh[:, ts(i, P)], qh[:, qs],
                                                 start=True, stop=True)
                                exp_sb = exp_pool.tile([P, SBLK], BF16,
                                                       tag="exp")
                                nc.scalar.activation(
                                    exp_sb[:, off:], sc_ps[:, off:],
                                    mybir.ActivationFunctionType.Exp,
                                    scale=SCALE)
                                if r >= 0:
                                    nc.vector.tensor_mul(
                                        out=exp_sb[:, off:],
                                        in0=exp_sb[:, off:],
                                        in1=masks_sb[:, r, off:])
                                # all-ones lhsT -> the k-sum lands in PSUM
                                # replicated across all 128 partitions
                                nc.tensor.matmul(
                                    den_ps[:, off:], ones_sb[:],
                                    exp_sb[:, off:],
                                    start=(i == 0), stop=(i == nkt - 1))
                                nc.tensor.matmul(
                                    av_ps[:, off:], vh[:, i, :],
                                    exp_sb[:, off:],
                                    start=(i == 0), stop=(i == nkt - 1))
                            recip_sb = rcp_pool.tile([P, SBLK], F32,
                                                     tag="rcp")
                            nc.vector.reciprocal_approx_fast(
                                out=recip_sb[:], in_=den_ps[:])
                            nc.vector.tensor_mul(
                                out=attnT_sb[:, h, ts(j, SBLK)],
                                in0=av_ps[:], in1=recip_sb[:])

                        if h % 2 == 1:
                            g = h // 2
                            nc.sync.dma_start(
                                ccins[g].ap().rearrange("(o p) s -> p o s",
                                                        p=P),
                                attnT_sb[:, h - 1:h + 1, :])
                            nc.gpsimd.collective_compute(
                                "AllGather", mybir.AluOpType.bypass,
                                replica_groups=[list(range(NCORES))],
                                ins=[ccins[g].ap()], outs=[ccouts[g].ap()],
                            )
                        if h == 3:
                            # stage pass 0/1 inputs; these SP triggers wait
                            # on AllGather 01, which is already in flight,
                            # and nothing later needs the SP queue soon
                            for ph in (0, 1):
                                load_slab(ph, 0)
                                load_slab(ph, 1)

                # ---------------- Phase 3: o_proj ------------------------
                with (
                    tc.tile_pool(name="osb", bufs=3) as osb_pool,
                    tc.tile_pool(name="part", bufs=1) as part_pool,
                    tc.tile_pool(name="ps_out", bufs=6, space="PSUM") as ps_out,
                ):
                    # Four passes, one per local head h: pass h needs only
                    # AllGather h (ccout[h] block c = global head 4c + h,
                    # i.e. wot f-tile 4c + h).  The whole gathered [1024, S]
                    # is loaded as one double-buffered slab per pass; slabs
                    # 0-2 were staged during attention, slab 3 here.
                    NST = S // P
                    part_sb = part_pool.tile([P, NST, ECOLS], F32)
                    for p_h in range(HPC):
                        for st in range(NST):
                            a_sb = slabs[(p_h, st // 8)]
                            o_ps = ps_out.tile([P, ECOLS], F32, tag="out")
                            for c in range(NCORES):
                                nc.tensor.matmul(o_ps[:],
                                                 a_sb[:, c, ts(st % 8, P)],
                                                 wot_sb[:, 4 * c + p_h, :],
                                                 start=(c == 0), stop=(c == 7))
                            if p_h == 0:
                                nc.any.tensor_copy(part_sb[:, st, :], o_ps[:])
                            elif p_h < HPC - 1:
                                nc.vector.tensor_add(out=part_sb[:, st, :],
                                                     in0=part_sb[:, st, :],
                                                     in1=o_ps[:])
                            else:
                                o_sb = osb_pool.tile([P, ECOLS], F32,
                                                     tag="osb")
                                nc.vector.tensor_add(out=o_sb[:], in0=o_ps[:],
                                                     in1=part_sb[:, st, :])
                                # halves on two queues to shorten the final
                                # store of the last tile
                                nc.sync.dma_start(
                                    out_ext.ap()[ts(st, P), :ECOLS // 2],
                                    o_sb[:, :ECOLS // 2])
                                nc.sync.dma_start(
                                    out_ext.ap()[ts(st, P), ECOLS // 2:],
                                    o_sb[:, ECOLS // 2:])
                            if st == 7 and p_h + 2 < HPC:
                                # stage the next half-slab into the buffer
                                # this half-pass just freed
                                load_slab(p_h + 2, 0)
                        if p_h + 2 < HPC:
                            load_slab(p_h + 2, 1)

    nc.compile()
    _NC_CACHE = nc
    return nc


def _prep_inputs(hidden_states, cos, sin, w_pack, w_o):
    hs = np.asarray(hidden_states, dtype=np.float32).reshape(S, E)
    xt = np.ascontiguousarray(hs.T).astype(BF16NP)        # [E, S]
    # pack x^T as [block, p, e-tile, s]: 8-32KB contiguous per partition
    xt_packed = np.ascontiguousarray(
        xt.reshape(ET, P, NSBLK, SBLK).transpose(2, 1, 0, 3))
    cost = np.ascontiguousarray(np.asarray(cos, dtype=np.float32).T)
    sint = np.ascontiguousarray(np.asarray(sin, dtype=np.float32).T)
    # rotate-half on device is an unsigned partition swap; the sign lives here
    sint = sint.copy()
    sint[:D // 2] = -sint[:D // 2]
    w_pack = np.asarray(w_pack, dtype=np.float32)
    w_o = np.asarray(w_o, dtype=np.float32)

    # swap permutation for the rope rotate matmul (symmetric, so rt == R)
    R = np.zeros((D, D), dtype=np.float32)
    half = D // 2
    R[np.arange(half), np.arange(half) + half] = 1.0
    R[np.arange(half) + half, np.arange(half)] = 1.0
    rt = np.ascontiguousarray(R).astype(BF16NP)

    masks = np.zeros((4, P, SBLK), dtype=np.float32)
    kk = np.arange(P)[:, None]
    qq = np.arange(SBLK)[None, :]
    for r in range(4):
        masks[r] = (P * r + kk <= qq).astype(np.float32)
    masks = masks.astype(BF16NP)

    ones = np.ones((P, P), dtype=BF16NP)

    in_maps = []
    hw = E // NCORES  # 512 head-rows per core in each of q/k/v
    for c in range(NCORES):
        rows = slice(c * hw, (c + 1) * hw)
        # q/k weights: [8, p, e-tile, 128] per-f-tile packed
        wqkT = np.concatenate([w_pack[rows], w_pack[E:][rows]], axis=0)
        wqkT = np.ascontiguousarray(wqkT.T).astype(BF16NP)   # [E, 1024]
        wqk_packed = np.ascontiguousarray(
            wqkT.reshape(ET, P, NQK, P).transpose(2, 1, 0, 3))
        # v weights: [p, e-tile, 512]
        wvT = np.ascontiguousarray(w_pack[2 * E:][rows].T).astype(BF16NP)
        wv_packed = np.ascontiguousarray(
            wvT.reshape(ET, P, VF).transpose(1, 0, 2))
        # o_proj weights: [p, e-tile, 512]
        wotT = np.ascontiguousarray(w_o[rows].T).astype(BF16NP)
        wot_packed = np.ascontiguousarray(
            wotT.reshape(ET, P, ECOLS).transpose(1, 0, 2))
        in_maps.append({
            "xt": xt_packed, "wqk": wqk_packed, "wv": wv_packed,
            "wot": wot_packed, "cost": cost, "sint": sint, "rt": rt,
            "masks": masks, "ones": ones,
        })
    return in_maps


def run(trace=False, trace_cores=None, **inputs):
    nc = build()
    in_maps = _prep_inputs(**inputs)
    res = run_bass_kernel_spmd(
        nc, in_maps, core_ids=list(range(NCORES)),
        trace=trace, trace_cores=trace_cores,
    )
    out = np.concatenate([res.results[c]["out"] for c in range(NCORES)], axis=1)
    return out.reshape(B, S, E).astype(np.float32), res


def kernel(**inputs) -> np.ndarray:
    out, _ = run(trace=False, **inputs)
    return out


# revision 47
# speedup vs baseline: 1.0307x; 1.0307x over previous
"""Baichuan attention layer (B=1, S=2048, E=4096, H=32, D=128) on 8 Trainium2
NeuronCores.

Sharding:
- QKV projection + RoPE + causal attention: tensor-parallel by head (4 heads
  per core).  q/k live in transposed [feature, seq] layout so the RoPE and
  score matmuls contract over the partition dim with zero transposes:
    qk^T[f, s]    = W @ X^T                (lhsT = W^T tiles, rhs = X^T tiles)
    scores^T[k,q] = K @ Q^T                (lhsT = K^T tile, rhs = Q^T block)
    att^T[d, q]   = V^T @ P^T              (lhsT = V tile,   rhs = exp tile)
  v is computed directly in [seq, (head d)] layout by swapping matmul
  operands (lhsT = X^T s-tile, rhs = Wv^T block) so no PE transposes are
  needed before attention.  Softmax runs without max-subtraction (scores ~
  N(0,1) after 1/sqrt(D), fp32 exp is safe); the denominator is accumulated
  with an all-ones [128,128] lhsT matmul so the k-sum lands in PSUM already
  replicated across partitions.  The causal-mask multiply runs on the (idle)
  GpSimd engine to keep the Vector queue short.
- One AllGather of att^T [512, 2048] bf16 per core -> full att^T [4096, 2048].
- o_proj: column-parallel (each core computes its 512 output columns for the
  full sequence, using its slice of w_o).  Host concatenates along E.

All matmuls in bf16 with fp32 PSUM accumulation.  All DRAM operands are
host-packed so every DMA line is 2KB-32KB contiguous per partition (the
naive layouts produced 256B descriptors that throttled the DMA queues).
"""

import importlib.util
import sys
import types

import numpy as np
import ml_dtypes

BF16NP = ml_dtypes.bfloat16

B, S, E = 1, 2048, 4096
H, D = 32, 128
NCORES = 8
HPC = H // NCORES          # heads per core = 4
P = 128                    # partitions
SBLK = 512                 # seq block (matmul free dim)
NSBLK = S // SBLK          # 4
ET = E // P                # 32 e-tiles
NQK = 2 * HPC              # 8 q/k f-tiles per core (q0..3, k0..3)
KT = S // P                # 16 k-tiles
ECOLS = E // NCORES        # 512 output columns per core
VF = HPC * D               # 512 v feature columns per core
SCALE = 1.0 / float(np.sqrt(D))


def _install_ntff_hook():
    """antenv.axon_hooks is absent in this image; recreate it from trn_boot's
    ctypes shim so run_bass_kernel_spmd(trace=True) can capture NTFF traces."""
    if "antenv.axon_hooks" in sys.modules:
        return
    try:
        spec = importlib.util.spec_from_file_location(
            "trn_boot", "/root/.axon_site/trn_agent_boot/trn_boot.py")
        tb = importlib.util.module_from_spec(spec)
        spec.loader.exec_module(tb)
        hook = tb._ntff_profile_via_ctypes("/opt/axon/libaxon_pjrt.so")
    except Exception:
        hook = None
    mod = types.ModuleType("antenv.axon_hooks")
    mod.get_axon_ntff_profile_hook = lambda: hook
    mod.set_axon_ntff_profile_hook = lambda h: None
    sys.modules["antenv.axon_hooks"] = mod


_install_ntff_hook()

import concourse.bass as bass  # noqa: E402
import concourse.mybir as mybir  # noqa: E402
import concourse.tile as tile  # noqa: E402
from concourse import bacc  # noqa: E402
from concourse.bass import ts  # noqa: E402
from concourse.bass_utils import run_bass_kernel_spmd  # noqa: E402

BF16 = mybir.dt.bfloat16
F32 = mybir.dt.float32

_NC_CACHE = None


def build():
    global _NC_CACHE
    if _NC_CACHE is not None:
        return _NC_CACHE
    nc = bacc.Bacc("TRN2", target_bir_lowering=False, debug=False,
                   num_devices=NCORES)

    # Host-packed layouts: partition dim explicit, per-partition lines long.
    xt_ext = nc.dram_tensor("xt", [NSBLK, P, ET, SBLK], BF16,
                            kind="ExternalInput")
    wqk_ext = nc.dram_tensor("wqk", [NQK, P, ET, P], BF16,
                             kind="ExternalInput")
    wv_ext = nc.dram_tensor("wv", [P, ET, VF], BF16, kind="ExternalInput")
    wot_ext = nc.dram_tensor("wot", [P, ET, ECOLS], BF16,
                             kind="ExternalInput")
    cost_ext = nc.dram_tensor("cost", [D, S], F32, kind="ExternalInput")
    # sin with rows 0..63 negated: rope is q*cos + swap(q)*sin_signed where
    # swap is the rt permutation matmul
    sint_ext = nc.dram_tensor("sint", [D, S], F32, kind="ExternalInput")
    rt_ext = nc.dram_tensor("rt", [D, D], BF16, kind="ExternalInput")
    masks_ext = nc.dram_tensor("masks", [4, P, SBLK], BF16,
                               kind="ExternalInput")
    ones_ext = nc.dram_tensor("ones", [P, P], BF16, kind="ExternalInput")
    out_ext = nc.dram_tensor("out", [S, ECOLS], F32, kind="ExternalOutput")

    # One AllGather per local head, issued as soon as that head's attention
    # output is ready: all four overlap attention/o_proj compute.  ccout[h]
    # rank-r block = rows [128r, 128r+128) = global head 4r + h.
    ccins = [nc.dram_tensor(f"ccin{h}", [P, S], BF16) for h in range(HPC)]
    ccouts = [nc.dram_tensor(f"ccout{h}", [NCORES * P, S], BF16,
                             addr_space="Shared") for h in range(HPC)]

    xt_t = xt_ext.ap().rearrange("b p e s -> p b e s")
    wqk_t = wqk_ext.ap().rearrange("f p e c -> p f e c")
    masks_t = masks_ext.ap().rearrange("r p q -> p r q")
    # [p, c, s]: block c of ccout[h] = global head 4c + h
    ccout_ts = [cc.ap().rearrange("(c p) s -> p c s", p=P) for cc in ccouts]

    with tile.TileContext(nc) as tc:
        with (
            tc.tile_pool(name="cst", bufs=1) as cst,
            tc.tile_pool(name="ropeT", bufs=1) as ropeT_pool,
            tc.tile_pool(name="vall", bufs=1) as vall_pool,
        ):
            rt_sb = cst.tile([D, D], BF16)
            ones_sb = cst.tile([P, P], BF16)
            masks_sb = cst.tile([P, 4, SBLK], BF16)

            # q^T and k^T after RoPE: [128, 8, 2048]
            ropeT_sb = ropeT_pool.tile([P, NQK, S], BF16)
            # V in [s, (h d)] layout: [128, 16 k-tiles, 512]
            v_all_sb = vall_pool.tile([P, KT, VF], BF16)

            # ---------------- Phase 1: QKV projection + RoPE -------------
            with (
                tc.tile_pool(name="xt", bufs=2) as xt_pool,
                tc.tile_pool(name="wqk", bufs=3) as w_pool,
                tc.tile_pool(name="wv", bufs=1) as wv_pool,
                tc.tile_pool(name="cs", bufs=1) as cs_pool,
                tc.tile_pool(name="qkc", bufs=3) as qkc_pool,
                tc.tile_pool(name="rtmp", bufs=3) as rtmp_pool,
                tc.tile_pool(name="ps_qkv", bufs=4, space="PSUM") as ps_qkv,
                tc.tile_pool(name="ps_rot", bufs=2, space="PSUM") as ps_rot,
                tc.tile_pool(name="ps_v", bufs=2, space="PSUM") as ps_v,
            ):
                cos_sb = cs_pool.tile([D, S], F32)
                sin_sb = cs_pool.tile([D, S], F32)
                wv_sb = wv_pool.tile([P, ET, VF], BF16)

                def v_groups(vb, xt_tile):
                    # v for block vb, directly in [s, (h d)] layout:
                    # out[s128, f512] = sum_e xt[e, s-tile].T @ wv[e-tile, :]
                    for st in range(NSBLK):
                        v_ps = ps_v.tile([P, VF], F32, tag="v")
                        for e in range(ET):
                            nc.tensor.matmul(
                                v_ps[:], xt_tile[:, e, ts(st, P)],
                                wv_sb[:, e, :],
                                start=(e == 0), stop=(e == ET - 1),
                            )
                        nc.any.tensor_copy(v_all_sb[:, 4 * vb + st, :],
                                           v_ps[:])

                prev_xt = None
                for b in range(NSBLK):
                    sblk = ts(b, SBLK)
                    xt_sb = xt_pool.tile([P, ET, SBLK], BF16, tag="xt")
                    w0_sb = None
                    if b == 0:
                        # tiny interleaved leading chunks: the first matmuls
                        # need only (xt e0-1, w0 e0-3), so the PE can start
                        # right after its preamble
                        w0_sb = w_pool.tile([P, ET, P], BF16, tag="w")
                        xs = [2, 2, 4, 4, 4, 4, 4, 4, 4]
                        wsz = [4, 4, 8, 16]
                        xo = [sum(xs[:i]) for i in range(len(xs))]
                        wo = [sum(wsz[:i]) for i in range(len(wsz))]
                        seq = []
                        for i in range(len(xs)):
                            seq.append(("x", xo[i], xs[i]))
                            if i < len(wsz):
                                seq.append(("w", wo[i], wsz[i]))
                        for kind, e0, sz in seq:
                            if kind == "x":
                                nc.sync.dma_start(
                                    xt_sb[:, bass.ds(e0, sz), :],
                                    xt_t[:, b, bass.ds(e0, sz), :])
                            else:
                                nc.sync.dma_start(
                                    w0_sb[:, bass.ds(e0, sz), :],
                                    wqk_t[:, 0, bass.ds(e0, sz), :])
                    else:
                        for ch in range(4):
                            nc.sync.dma_start(xt_sb[:, ts(ch, 8), :],
                                              xt_t[:, b, ts(ch, 8), :])

                    for f in range(NQK):
                        if b == 0 and f == 0:
                            w_sb = w0_sb
                        else:
                            w_sb = w_pool.tile([P, ET, P], BF16, tag="w")
                            for ch in range(2):
                                nc.sync.dma_start(
                                    w_sb[:, ts(ch, 16), :],
                                    wqk_t[:, f, ts(ch, 16), :])
                        if b == 0 and f == 0:
                            # only what the first rope mul needs; wv and the
                            # attention consts load in later, calmer windows
                            nc.sync.dma_start(rt_sb[:], rt_ext.ap())
                            nc.sync.dma_start(cos_sb[:], cost_ext.ap())
                            nc.sync.dma_start(sin_sb[:], sint_ext.ap())
                        if b == 0 and f == 4:
                            for ch in range(4):
                                nc.sync.dma_start(
                                    wv_sb[:, ts(ch, ET // 4), :],
                                    wv_ext.ap()[:, ts(ch, ET // 4), :])
                        if b == 2 and f == 0:
                            nc.sync.dma_start(ones_sb[:], ones_ext.ap())
                            nc.sync.dma_start(masks_sb[:], masks_t)
                        acc_ps = ps_qkv.tile([P, SBLK], F32, tag="qkv")
                        for e in range(ET):
                            nc.tensor.matmul(
                                acc_ps[:], w_sb[:, e, :], xt_sb[:, e, :],
                                start=(e == 0), stop=(e == ET - 1),
                            )
                        # RoPE: rot = R @ qk on the PE (sign of rotate-half
                        # folded into sint on the host; rt is a pure swap
                        # permutation)
                        qk_sb = qkc_pool.tile([P, SBLK], BF16, tag="qkc")
                        nc.any.tensor_copy(qk_sb[:], acc_ps[:])
                        rot_ps = ps_rot.tile([P, SBLK], F32, tag="rot")
                        nc.tensor.matmul(rot_ps[:], rt_sb[:], qk_sb[:],
                                         start=True, stop=True)
                        t1 = rtmp_pool.tile([P, SBLK], F32, tag="t1")
                        nc.vector.tensor_mul(out=t1[:], in0=acc_ps[:],
                                             in1=cos_sb[:, sblk])
                        t2 = rtmp_pool.tile([P, SBLK], F32, tag="t2")
                        nc.vector.tensor_mul(out=t2[:], in0=rot_ps[:],
                                             in1=sin_sb[:, sblk])
                        nc.vector.tensor_add(
                            out=ropeT_sb[:, f, sblk], in0=t1[:], in1=t2[:])

                    # v of the PREVIOUS block: its xt is still resident and
                    # this keeps block 0's startup window free of wv traffic
                    if b > 0:
                        v_groups(b - 1, prev_xt)
                    if b == NSBLK - 1:
                        v_groups(b, xt_sb)
                    prev_xt = xt_sb

            # ------------- Phase 2: attention per head + AllGather -------
            with (
                tc.tile_pool(name="wot", bufs=1) as wot_pool,
                tc.tile_pool(name="attnT", bufs=1) as attnT_pool,
                tc.tile_pool(name="slab", bufs=4) as slab_pool,
            ):
                wot_sb = wot_pool.tile([P, ET, ECOLS], BF16)
                for ch in range(4):
                    nc.sync.dma_start(wot_sb[:, ts(ch, ET // 4), :],
                                      wot_ext.ap()[:, ts(ch, ET // 4), :])
                attnT_sb = attnT_pool.tile([P, HPC, S], BF16)

                # o_proj input staged as half-slabs (st 0-7 / st 8-15 of one
                # gathered head-group) so the WAR gate on buffer reuse opens
                # at half-pass granularity and transfers start sooner
                slabs = {}

                def load_slab(p_h, half):
                    a_sb = slab_pool.tile([P, NCORES, S // 2], BF16,
                                          tag="slab")
                    s0 = half * (S // 2)
                    for cc in range(4):
                        nc.sync.dma_start(
                            a_sb[:, bass.ds(2 * cc, 2), :],
                            ccout_ts[p_h][:, bass.ds(2 * cc, 2),
                                          bass.ds(s0, S // 2)])
                    slabs[(p_h, half)] = a_sb

                with (
                    tc.tile_pool(name="exp", bufs=8) as exp_pool,
                    tc.tile_pool(name="rcp", bufs=2) as rcp_pool,
                    tc.tile_pool(name="ps_sc", bufs=4, space="PSUM") as ps_sc,
                    tc.tile_pool(name="ps_av", bufs=2, space="PSUM") as ps_av,
                    tc.tile_pool(name="ps_den", bufs=2, space="PSUM") as ps_den,
                ):
                    for h in range(HPC):
                        qh = ropeT_sb[:, h, :]
                        kh = ropeT_sb[:, HPC + h, :]
                        vh = v_all_sb[:, :, ts(h, D)]
                        for j in range(NSBLK):
                            nkt = 4 * j + 4
                            av_ps = ps_av.tile([P, SBLK], F32, tag="av")
                            den_ps = ps_den.tile([P, SBLK], F32, tag="den")
                            for i in range(nkt):
                                # diagonal tile r: columns below 128r are
                                # fully masked -> compute only [off:SBLK]
                                r = i - 4 * j
                                off = 128 * r if r > 0 else 0
                                qs = bass.ds(j * SBLK + off, SBLK - off)
                                sc_ps = ps_sc.tile([P, SBLK], F32, tag="sc")
                                nc.tensor.matmul(sc_ps[:, off:],
                                                 kh[:, ts(i, P)], qh[:, qs],
                                                 start=True, stop=True)
                                exp_sb = exp_pool.tile([P, SBLK], BF16,
                                                       tag="exp")
                                nc.scalar.activation(
                                    exp_sb[:, off:], sc_ps[:, off:],
                                    mybir.ActivationFunctionType.Exp,
                                    scale=SCALE)
                                if r >= 0:
                                    nc.vector.tensor_mul(
                                        out=exp_sb[:, off:],
                                        in0=exp_sb[:, off:],
                                        in1=masks_sb[:, r, off:])
                                # all-ones lhsT -> the k-sum lands in PSUM
                                # replicated across all 128 partitions
                                nc.tensor.matmul(
                                    den_ps[:, off:], ones_sb[:],
                                    exp_sb[:, off:],
                                    start=(i == 0), stop=(i == nkt - 1))
                                nc.tensor.matmul(
                                    av_ps[:, off:], vh[:, i, :],
                                    exp_sb[:, off:],
                                    start=(i == 0), stop=(i == nkt - 1))
                            recip_sb = rcp_pool.tile([P, SBLK], F32,
                                                     tag="rcp")
                            nc.vector.reciprocal_approx_fast(
                                out=recip_sb[:], in_=den_ps[:])
                            nc.vector.tensor_mul(
                                out=attnT_sb[:, h, ts(j, SBLK)],
                                in0=av_ps[:], in1=recip_sb[:])

                        nc.sync.dma_start(
                            ccins[h].ap().rearrange("(o p) s -> p o s", p=P),
                            attnT_sb[:, h:h + 1, :])
                        nc.gpsimd.collective_compute(
                            "AllGather", mybir.AluOpType.bypass,
                            replica_groups=[list(range(NCORES))],
                            ins=[ccins[h].ap()], outs=[ccouts[h].ap()],
                        )
                        if h >= 2:
                            # at most 4 half-slabs staged before phase 3
                            # consumes them in order
                            load_slab(h - 2, 0)
                            load_slab(h - 2, 1)

                # ---------------- Phase 3: o_proj ------------------------
                with (
                    tc.tile_pool(name="osb", bufs=3) as osb_pool,
                    tc.tile_pool(name="part", bufs=1) as part_pool,
                    tc.tile_pool(name="ps_out", bufs=6, space="PSUM") as ps_out,
                ):
                    # Four passes, one per local head h: pass h needs only
                    # AllGather h (ccout[h] block c = global head 4c + h,
                    # i.e. wot f-tile 4c + h).  The whole gathered [1024, S]
                    # is loaded as one double-buffered slab per pass; slabs
                    # 0-2 were staged during attention, slab 3 here.
                    NST = S // P
                    part_sb = part_pool.tile([P, NST, ECOLS], F32)
                    for p_h in range(HPC):
                        for st in range(NST):
                            a_sb = slabs[(p_h, st // 8)]
                            o_ps = ps_out.tile([P, ECOLS], F32, tag="out")
                            for c in range(NCORES):
                                nc.tensor.matmul(o_ps[:],
                                                 a_sb[:, c, ts(st % 8, P)],
                                                 wot_sb[:, 4 * c + p_h, :],
                                                 start=(c == 0), stop=(c == 7))
                            if p_h == 0:
                                nc.any.tensor_copy(part_sb[:, st, :], o_ps[:])
                            elif p_h < HPC - 1:
                                nc.vector.tensor_add(out=part_sb[:, st, :],
                                                     in0=part_sb[:, st, :],
                                                     in1=o_ps[:])
                            else:
                                o_sb = osb_pool.tile([P, ECOLS], F32,
                                                     tag="osb")
                                nc.vector.tensor_add(out=o_sb[:], in0=o_ps[:],
                                                     in1=part_sb[:, st, :])
                                # halves on two queues to shorten the final
                                # store of the last tile
                                nc.sync.dma_start(
                                    out_ext.ap()[ts(st, P), :ECOLS // 2],
                                    o_sb[:, :ECOLS // 2])
                                nc.sync.dma_start(
                                    out_ext.ap()[ts(st, P), ECOLS // 2:],
                                    o_sb[:, ECOLS // 2:])
                            if st == 7 and p_h + 2 < HPC:
                                # stage the next half-slab into the buffer
                                # this half-pass just freed
                                load_slab(p_h + 2, 0)
                        if p_h + 2 < HPC:
                            load_slab(p_h + 2, 1)

    nc.compile()
    _NC_CACHE = nc
    return nc


def _prep_inputs(hidden_states, cos, sin, w_pack, w_o):
    hs = np.asarray(hidden_states, dtype=np.float32).reshape(S, E)
    xt = np.ascontiguousarray(hs.T).astype(BF16NP)        # [E, S]
    # pack x^T as [block, p, e-tile, s]: 8-32KB contiguous per partition
    xt_packed = np.ascontiguousarray(
        xt.reshape(ET, P, NSBLK, SBLK).transpose(2, 1, 0, 3))
    cost = np.ascontiguousarray(np.asarray(cos, dtype=np.float32).T)
    sint = np.ascontiguousarray(np.asarray(sin, dtype=np.float32).T)
    # rotate-half on device is an unsigned partition swap; the sign lives here
    sint = sint.copy()
    sint[:D // 2] = -sint[:D // 2]
    w_pack = np.asarray(w_pack, dtype=np.float32)
    w_o = np.asarray(w_o, dtype=np.float32)

    # swap permutation for the rope rotate matmul (symmetric, so rt == R)
    R = np.zeros((D, D), dtype=np.float32)
    half = D // 2
    R[np.arange(half), np.arange(half) + half] = 1.0
    R[np.arange(half) + half, np.arange(half)] = 1.0
    rt = np.ascontiguousarray(R).astype(BF16NP)

    masks = np.zeros((4, P, SBLK), dtype=np.float32)
    kk = np.arange(P)[:, None]
    qq = np.arange(SBLK)[None, :]
    for r in range(4):
        masks[r] = (P * r + kk <= qq).astype(np.float32)
    masks = masks.astype(BF16NP)

    ones = np.ones((P, P), dtype=BF16NP)

    in_maps = []
    hw = E // NCORES  # 512 head-rows per core in each of q/k/v
    for c in range(NCORES):
        rows = slice(c * hw, (c + 1) * hw)
        # q/k weights: [8, p, e-tile, 128] per-f-tile packed
        wqkT = np.concatenate([w_pack[rows], w_pack[E:][rows]], axis=0)
        wqkT = np.ascontiguousarray(wqkT.T).astype(BF16NP)   # [E, 1024]
        wqk_packed = np.ascontiguousarray(
            wqkT.reshape(ET, P, NQK, P).transpose(2, 1, 0, 3))
        # v weights: [p, e-tile, 512]
        wvT = np.ascontiguousarray(w_pack[2 * E:][rows].T).astype(BF16NP)
        wv_packed = np.ascontiguousarray(
            wvT.reshape(ET, P, VF).transpose(1, 0, 2))
        # o_proj weights: [p, e-tile, 512]
        wotT = np.ascontiguousarray(w_o[rows].T).astype(BF16NP)
        wot_packed = np.ascontiguousarray(
            wotT.reshape(ET, P, ECOLS).transpose(1, 0, 2))
        in_maps.append({
            "xt": xt_packed, "wqk": wqk_packed, "wv": wv_packed,
            "wot": wot_packed, "cost": cost, "sint": sint, "rt": rt,
            "masks": masks, "ones": ones,
        })
    return in_maps


def run(trace=False, trace_cores=None, **inputs):
    nc = build()
    in_maps = _prep_inputs(**inputs)
    res = run_bass_kernel_spmd(
        nc, in_maps, core_ids=list(range(NCORES)),
        trace=trace, trace_cores=trace_cores,
    )
    out = np.concatenate([res.results[c]["out"] for c in range(NCORES)], axis=1)
    return out.reshape(B, S, E).astype(np.float32), res


def kernel(**inputs) -> np.ndarray:
    out, _ = run(trace=False, **inputs)
    return out
